# revision 58
# baseline (speedup 1.0000x reference)
"""AttentionPool (segment softmax-pool) Trainium2 kernel, 8 NeuronCores.

Math (reference):
    s = tanh(x @ W1 + b1) @ W2 + b2        # [N,1] scores
    e = exp(s - max(s))                    # global max shift
    out[b] = sum_{i in seg b} e_i x_i / (sum_{i in seg b} e_i + 1e-8)

Key identity: the global max shift cancels in the ratio (up to the
negligible 1e-8 term; |s| <= ||W2||_1 ~ 11 so exp never overflows), so we
compute e = exp(s) directly.  Every row's contribution is then local, and
with batch ids sorted, segments are contiguous runs.  Core c owns segments
[128c, 128(c+1)) and processes a fixed window of F rows starting at the
first row of segment 128c.  Rows of other cores' segments inside the
window self-mask: their relative id falls outside [0,128) so the one-hot
compare produces zero columns.

Numerics: pooling path in bf16; the shipped-transpose score path runs at
fp8 e3m4 (x and W1*32, compensated via tanh's input scale) - device-
measured end-to-end max rel err 1.60e-2 vs the 2e-2 gate (deterministic
for the fixed harness input).  e4m3 anywhere fails the gate (x-pool
3.9e-2, score-path 2.7e-2 even with weight prescaling); x-pool must
stay 2-byte.
The host ships TWO layouts of the window: row-major xr [F, D] bf16
(pooling matmul moving operand, DMA'd one block per DMA with 4
rows/partition so every partition is one contiguous 4KB descriptor; the
implied row permutation is folded into brel/xtb host-side) and
block-contiguous pre-transposed xtb [blk, 128, 4, 512] fp8e3 ([blk, p,
k, r] = x[512 blk + r, 128k + p], the W1 matmul moving operand, 2KB
/partition descriptors).  Scheduled blocks skip the xtb DMA and instead
PE-transpose the row-major tiles (PSUM bf16 -> one DVE copy to SBUF),
balancing the DMA engines against the PE.

    per 128-row tile on device (bf16 matmuls, 1 col/cycle @2.4GHz):
      u    = W1_k.T @ xts_k  (accum over k)            # [128h, 512r]
      th   = tanh(u + b1)  (bf16)                      # ACT
      s    = th.T @ W2 ; e = exp(s + b2)  (bf16)       # PE + ACT
      A    = (iota == brel) * e  (bf16)                # DVE
      num += A.T @ xr ; den += A.T @ ones_col          # PSUM f32 accum
    out = num * 1/(den + 1e-8), one [128,512] slab per core; host concat.

e is scalar-departitioned once per GB-block group via a DRAM bounce with
contiguous descriptors ([1, gn] -> [gt, 128]) plus a tiny PE transpose
to [128, gt] (the AP balancer cannot split partition 0 into 128
partitions in one hop, and a direct strided departition DMA costs 2048
4-byte descriptors ~ 5-9us of sequencer time per group).

Engine-queue scheduling: instructions execute in emission order per
engine, and the PE p-state model runs ~2x slower for the first 3us after
any idle gap, so emission is software-pipelined across blocks
(transposes for block b, W1 for b-1, score for b-2, pass2 num/den
matmuls lagged behind the bounce with A-builds one step ahead) so every
instruction's inputs are at least one block old when the engine reaches
it.  The dev/ship choice is time-scheduled (first 6 blocks dev while
the DMA queue warms up, last 8 ship so the PE-bound tail has no
transposes, 8 spread through the middle).  TimelineSim: 175.1us vs the
299.2us baseline (harness-measured 319.6us, sim tracked it within 7%).
"""

import os
import sys

for _p in ("/opt/trn_rl_repo",):
    if os.path.isdir(_p) and _p not in sys.path:
        sys.path.append(_p)

import numpy as np
import ml_dtypes

N_CORES = 8
B = 1024
SEGS = B // N_CORES          # 128 segments owned per core
D = 512
H = 128
F = 33792                    # fixed per-core row window (264 tiles of 128)
TILES = F // 128
DEV_NUM, DEV_DEN = 4, 9      # fraction of blocks transposed on-device


def build_nc(tiles=TILES, repeats=1, bufs=None, gb=8,
             dev_num=DEV_NUM, dev_den=DEV_DEN, ship_lead=2, oct_lead=2,
             lag=2, prefetch_at_end=False, dev_spread=False, tail_gb=4,
             ship_alt=False, x_alt=False,
             dev_sched=(6, 14, 12)):
    """Build the per-core Bass program. dev_num/dev_den: fraction of 4-tile
    blocks whose transposed layout is built on-device (PE transpose) rather
    than DMA'd from the host-shipped xtb. repeats>1 re-emits the whole
    computation for delta-timing."""
    bufs = {**dict(x=14, xts=6, th=4, A=8, es=3, xtp=2, u=2, e=2),
            **(bufs or {})}
    import concourse.bacc as bacc
    import concourse.mybir as mybir
    import concourse.tile as tile

    F32 = mybir.dt.float32
    F32R = mybir.dt.float32r
    BF16 = mybir.dt.bfloat16
    AF = mybir.ActivationFunctionType
    OP = mybir.AluOpType

    f_rows = tiles * 128
    assert tiles % 4 == 0
    nblk = tiles // 4

    nc = bacc.Bacc(None, target_bir_lowering=False)
    cst_d = nc.dram_tensor("cst", (128, 257), BF16, kind="ExternalInput")
    xr_d = nc.dram_tensor("xr", (f_rows, D), BF16, kind="ExternalInput")
    FP8E3 = __import__('concourse.mybir', fromlist=['dt']).dt.float8e3
    xtb_d = nc.dram_tensor("xtb", (nblk, 128, 4, 512), FP8E3,
                           kind="ExternalInput")
    w18_d = nc.dram_tensor("w18", (128, 4, H), FP8E3, kind="ExternalInput")
    brel_d = nc.dram_tensor("brel", (128, tiles), F32, kind="ExternalInput")
    w1_d = nc.dram_tensor("w1", (128, 4, H), BF16, kind="ExternalInput")
    w2_d = nc.dram_tensor("w2", (H, 2), BF16, kind="ExternalInput")
    b1_d = nc.dram_tensor("b1", (H, 1), F32, kind="ExternalInput")
    b2_d = nc.dram_tensor("b2", (128, 1), F32, kind="ExternalInput")
    out_d = nc.dram_tensor("out", (SEGS, D), F32, kind="ExternalOutput")

    def is_dev_block(blk):
        # dev_num of every dev_den blocks are transposed on-device
        if dev_sched is not None:
            if len(dev_sched) == 4:
                front, total, tail_s, lead_ship = dev_sched
            else:
                front, total, tail_s = dev_sched
                lead_ship = 0
            if blk < lead_ship:
                return False
            if blk < front + lead_ship:
                return True
            blk = blk - lead_ship
            last = nblk - tail_s - lead_ship
            if blk >= last:
                return False
            mid = last - front
            need = total - front
            r = blk - front
            return (r * need) // mid != ((r + 1) * need) // mid
        if dev_spread:
            r = blk % dev_den
            return (r * dev_num) // dev_den != ((r + 1) * dev_num) // dev_den
        return (blk % dev_den) < dev_num

    with tile.TileContext(nc) as tc:
        with (
            tc.tile_pool(name="const", bufs=1) as cpool,
            tc.tile_pool(name="xin", bufs=bufs["x"]) as xpool,
            tc.tile_pool(name="xts", bufs=bufs["xts"]) as xtspool,
            tc.tile_pool(name="xts8", bufs=bufs["xts"]) as xts8pool,
            tc.tile_pool(name="th", bufs=bufs["th"]) as thpool,
            tc.tile_pool(name="abuild", bufs=bufs["A"]) as apool,
            tc.tile_pool(name="esb", bufs=bufs["es"]) as espool,
            tc.tile_pool(name="e4", bufs=bufs["es"]) as e4pool,
            tc.tile_pool(name="edram", bufs=bufs["es"], space="DRAM") as dramp,
            tc.tile_pool(name="fin", bufs=1) as fpool,
            tc.tile_pool(name="ps_xt", bufs=bufs["xtp"], space="PSUM") as xtpsum,
            tc.tile_pool(name="ps_u", bufs=bufs["u"], space="PSUM") as upsum,
            tc.tile_pool(name="ps_e", bufs=bufs["e"], space="PSUM") as epsum,
            tc.tile_pool(name="ps_num", bufs=1, space="PSUM") as numpsum,
            tc.tile_pool(name="ps_den", bufs=1, space="PSUM") as denpsum,
        ):
            # ---- constants ----
            w1b = cpool.tile([128, 4, H], BF16)
            nc.sync.dma_start(w1b[:], w1_d[:])
            w18 = cpool.tile([128, 4, H], FP8E3)
            nc.sync.dma_start(w18[:], w18_d[:])
            w2r = cpool.tile([H, 2], BF16)
            nc.scalar.dma_start(w2r[:], w2_d[:])
            b1s = cpool.tile([H, 1], F32)
            nc.scalar.dma_start(b1s[:], b1_d[:])
            b2s = cpool.tile([128, 1], F32)
            nc.scalar.dma_start(b2s[:], b2_d[:])
            brel = cpool.tile([128, tiles], F32)
            brel_loaded = [False]

            # host-shipped constants: [identity | iota-row | ones] bf16
            cst = cpool.tile([128, 257], BF16)
            nc.sync.dma_start(cst[:], cst_d[:])
            identb = cst[:, 0:128]     # identity for PE transposes
            iifb = cst[:, 128:256]     # iifb[p, j] = j, for the A build
            onesc = cst[:, 256:257]

            num = numpsum.tile([SEGS, D], F32)
            denc = denpsum.tile([SEGS, 1], F32)

            GB = gb  # blocks per departition group
            LAG = lag  # extra blocks between a group's bounce and its pass2
            nblk_b = tiles // 4
            copy_alt = [0]
            for rep in range(repeats):
                x8s = {}             # octet index -> tile (kept for pass2)
                xtss = {}            # block -> xts tile (kept one block)
                ths = {}             # block -> th tile (kept one block)
                ess = {}             # group -> es1 tile
                pend_blocks = []     # bases of blocks awaiting pass2 grouping
                grp_start = [0]      # first block of the current group
                p2q = []             # FIFO of pending pass2 groups

                def emit_abuild_block(grp):
                    """A-builds (DVE) for one block of a group, one pipeline
                    step ahead of its num/den matmuls.  First call also emits
                    the tiny e-departition transpose."""
                    if grp["e4"] is None:
                        gt = grp["gt"]
                        e4p = epsum.tile([128, GB * 4], BF16, tag="e")
                        nc.tensor.transpose(
                            e4p[:, 0:gt], grp["em"][0:gt, :],
                            identb[0:gt, 0:gt],
                        )
                        e4 = e4pool.tile([128, GB * 4], F32, tag="e4")
                        nc.vector.tensor_copy(e4[:, 0:gt], e4p[:, 0:gt])
                        grp["e4"] = e4
                    base2 = grp["blocks"].pop(0)
                    A4 = apool.tile([128, 4, SEGS], BF16, tag="A")
                    for q2 in range(4):
                        tt = base2 + q2
                        ecol = tt - 4 * grp["start"]
                        nc.vector.tensor_scalar(
                            A4[:, q2, :], iifb[:], brel[:, tt:tt + 1],
                            grp["e4"][:, ecol:ecol + 1],
                            op0=OP.is_equal, op1=OP.mult,
                        )
                    return (base2, A4)

                def emit_pass2_block(built):
                    """num/den matmuls for a block whose A was built one
                    pipeline step earlier."""
                    base2, A4 = built
                    for q2 in range(4):
                        tt = base2 + q2
                        nc.tensor.matmul(
                            num[:], A4[:, q2, :],
                            x8s[tt // 4][:, tt % 4, :],
                            start=(tt == 0), stop=(tt == tiles - 1),
                            skip_group_check=True,
                        )
                        # den rides the same stationary A: one extra column
                        nc.tensor.matmul(
                            denc[:], A4[:, q2, :], onesc[:],
                            start=(tt == 0), stop=(tt == tiles - 1),
                            skip_group_check=True,
                        )
                    done = [o for o in x8s if (o + 1) * 4 <= base2 + 4]
                    for o in done:
                        del x8s[o]

                built_q = []   # A-built pass2 blocks awaiting matmuls
                noct = tiles // 8

                def emit_prefetch(bi):
                    # x8 octets oct_lead octets ahead, ship DMAs ship_lead
                    # blocks ahead of their W1 stage
                    for blk_i in (list(range(2 * oct_lead + 1))
                                  if bi == 0 else [bi + 2 * oct_lead]):
                        if blk_i < nblk_b:
                            t0 = blk_i * 4
                            # one DMA per block; partition p holds 4 window
                            # rows 512g+4p..4p+3 as one contiguous 4KB
                            # descriptor (host permutes brel/xtb to match)
                            x4 = xpool.tile([128, 4, D], BF16, tag="x")
                            xq_eng = (nc.scalar if (x_alt and blk_i % 2 == 1)
                                      else nc.sync)
                            xq_eng.dma_start(
                                x4[:],
                                xr_d[t0 * 128:(t0 + 4) * 128, :].rearrange(
                                    "(p a) d -> p a d", p=128),
                            )
                            x8s[blk_i] = x4
                    for lead_bi in (list(range(ship_lead + 1)) if bi == 0
                                    else [bi + ship_lead]):
                        if lead_bi < nblk_b:
                            if is_dev_block(lead_bi):
                                xts = xtspool.tile([128, 4, D], BF16,
                                                   tag="xts", name="xts")
                            else:
                                xts = xts8pool.tile([128, 4, D], FP8E3,
                                                    tag="xts8", name="xts")
                                shipq = (nc.sync if (ship_alt and
                                                     lead_bi % 2 == 0)
                                         else nc.scalar)
                                shipq.dma_start(xts[:], xtb_d[lead_bi])
                            xtss[lead_bi] = xts

                if prefetch_at_end:
                    emit_prefetch(0)
                    if not brel_loaded[0]:
                        nc.scalar.dma_start(brel[:], brel_d[:])
                        brel_loaded[0] = True
                for bi in range(nblk_b + 3):
                    if not prefetch_at_end:
                        emit_prefetch(bi)
                    if not brel_loaded[0]:
                        nc.scalar.dma_start(brel[:], brel_d[:])
                        brel_loaded[0] = True
                    if bi < nblk_b:
                        for q in range(4):
                            t = 4 * bi + q
                            if is_dev_block(bi):
                                xt = x8s[t // 4][:, t % 4, :]
                                xq = xtpsum.tile([128, D], BF16, tag="xtp")
                                for k in range(4):
                                    nc.tensor.transpose(
                                        xq[:, k * 128:(k + 1) * 128],
                                        xt[:, k * 128:(k + 1) * 128],
                                        identb[:],
                                    )
                                dst = xtss[bi][:, :, q * 128:(q + 1) * 128]
                                src = xq[:].rearrange("p (k r) -> p k r", k=4)
                                nc.vector.tensor_copy(dst, src)
                                copy_alt[0] += 1

                    # ---- stage 1 (block bi-1): W1 matmuls + tanh
                    b1i = bi - 1
                    if 0 <= b1i < nblk_b:
                        nb = 4 * 128
                        xts1 = xtss.pop(b1i)
                        dev1 = is_dev_block(b1i)
                        w1use = w1b if dev1 else w18
                        u = upsum.tile([H, 4 * 128], F32, tag="u")
                        for k in range(4):
                            nc.tensor.matmul(
                                u[:, 0:nb],
                                w1use[:, k, :],
                                xts1[:, k, 0:nb],
                                start=(k == 0),
                                stop=(k == 3),
                            )
                        th = thpool.tile([H, 4 * 128], BF16, tag="th")
                        # ship blocks run the MLP at fp8e3 with W1
                        # pre-scaled x32; tanh's input scale compensates
                        nc.scalar.activation(
                            th[:, 0:nb], u[:, 0:nb],
                            AF.Tanh, bias=b1s[:],
                            scale=(1.0 if dev1 else 1.0 / 32.0),
                        )
                        ths[b1i] = th

                    # ---- stage 2 (block bi-2): score matmul + exp + bounce
                    b2i = bi - 2
                    if 0 <= b2i < nblk_b:
                        nb = 4 * 128
                        th2 = ths.pop(b2i)
                        ep2 = epsum.tile([2, 4 * 128], F32, tag="e")
                        nc.tensor.matmul(
                            ep2[:, 0:nb], w2r[:], th2[:, 0:nb],
                            start=True, stop=True, skip_group_check=True,
                        )
                        g = b2i - grp_start[0]      # slot within group
                        if g == 0:
                            es1 = espool.tile([1, GB * 512], BF16,
                                              tag="es", name="es1")
                            ess[0] = es1
                        es1 = ess[0]
                        nc.scalar.activation(
                            es1[0:1, g * 512:g * 512 + nb], ep2[0:1, 0:nb],
                            AF.Exp, bias=b2s[0:1, :], scale=1.0,
                        )
                        pend_blocks.append(4 * b2i)
                        rem_after = nblk_b - 1 - b2i
                        gb_cur = GB if (tail_gb == 0
                                        or rem_after >= GB + 2) else tail_gb
                        last_of_group = (g == gb_cur - 1) or (b2i == nblk_b - 1)
                        if last_of_group:
                            gs = grp_start[0]
                            grp_start[0] = b2i + 1
                            gn = g * 512 + nb    # valid scalars in group
                            gt = (gn + 127) // 128
                            # departition e: [1, gn] -> [gt, 128] via a DRAM
                            # bounce with contiguous descriptors; the tiny
                            # PE transpose to [128, gt] is emitted lagged in
                            # emit_pass2_block so PE never waits on it
                            ed = dramp.tile([GB * 512], BF16, tag="ed")
                            nc.scalar.dma_start(ed[0:gn], es1[0:1, 0:gn])
                            em = e4pool.tile([GB * 4, 128], BF16, tag="em")
                            nc.scalar.dma_start(
                                em[0:gt, :],
                                ed[0:gn].rearrange("(a u) -> a u", a=gt),
                            )
                            del ess[0]
                            p2q.append(dict(blocks=pend_blocks, em=em,
                                            gt=gt, e4=None, enq=bi, start=gs))
                            pend_blocks = []

                    # ---- lagged pass2, two sub-stages one step apart:
                    # num/den matmuls for the block A-built last step, then
                    # A-builds (DVE) for the next block
                    if built_q:
                        emit_pass2_block(built_q.pop(0))
                    if p2q and bi - p2q[0]["enq"] >= LAG:
                        built_q.append(emit_abuild_block(p2q[0]))
                        if not p2q[0]["blocks"]:
                            p2q.pop(0)
                    if prefetch_at_end:
                        emit_prefetch(bi + 1)
                # drain remaining pass2 groups
                while p2q or built_q:
                    if built_q:
                        emit_pass2_block(built_q.pop(0))
                    if p2q:
                        built_q.append(emit_abuild_block(p2q[0]))
                        if not p2q[0]["blocks"]:
                            p2q.pop(0)

                dsb = fpool.tile([SEGS, 1], F32)
                nc.vector.tensor_scalar(dsb[:], denc[:, 0:1],
                                        1e-8, None, op0=OP.add)
                rec = fpool.tile([SEGS, 1], F32)
                nc.vector.reciprocal(rec[:], dsb[:])
                osb = fpool.tile([SEGS, D], F32)
                nc.vector.tensor_scalar(osb[:], num[:], rec[:], None,
                                        op0=OP.mult)
                nc.sync.dma_start(out_d[:], osb[:])

    nc.compile()
    return nc


_NC_CACHE = {}


def get_nc(tiles=TILES):
    if tiles not in _NC_CACHE:
        _NC_CACHE[tiles] = build_nc(tiles)
    return _NC_CACHE[tiles]


def make_in_maps(x, batch, W1, b1, W2, b2, tiles=TILES, n_cores=N_CORES):
    """Host-side sharding: segment-aligned fixed windows + relative ids,
    both bf16 layouts of each window, pre-arranged W1.

    The device loads xr with 8 rows per partition per DMA (one contiguous
    8KB descriptor per partition): window position j = 1024g + 128a + p
    holds original window row 1024g + 8p + a.  brel and xtb are built in
    this permuted order so everything downstream stays consistent."""
    x = np.ascontiguousarray(np.asarray(x, dtype=np.float32))
    batch = np.asarray(batch).astype(np.int64)
    W1 = np.asarray(W1, dtype=np.float32)
    b1 = np.asarray(b1, dtype=np.float32).reshape(H, 1)
    W2 = np.ascontiguousarray(
        np.repeat(np.asarray(W2, dtype=np.float32).reshape(H, 1), 2, axis=1)
    ).astype(ml_dtypes.bfloat16)
    b2v = float(np.asarray(b2, dtype=np.float32).reshape(-1)[0])
    b2a = np.full((128, 1), b2v, dtype=np.float32)
    # w1 rearranged: [p, k, h] = W1[128k + p, h]; w18 = fp8e3 of W1*32
    w1p = np.ascontiguousarray(W1.reshape(4, 128, H).transpose(1, 0, 2))
    w1r = w1p.astype(ml_dtypes.bfloat16)
    w18 = (w1p * 32.0).astype(ml_dtypes.float8_e3m4)
    # device constants: [identity | iota-row | ones] bf16
    cst = np.zeros((128, 257), dtype=ml_dtypes.bfloat16)
    cst[:, 0:128] = np.eye(128, dtype=np.float32)
    cst[:, 128:256] = np.arange(128, dtype=np.float32)[None, :]
    cst[:, 256] = 1.0

    n = x.shape[0]
    f_rows = tiles * 128
    nblk = tiles // 4
    # perm[j] = original window row at permuted position j
    perm = (np.arange(f_rows).reshape(-1, 128, 4)
            .transpose(0, 2, 1).reshape(f_rows))
    bounds = np.searchsorted(batch, np.arange(0, n_cores + 1) * SEGS)
    owned = np.diff(bounds)
    if owned.max() > f_rows:
        return None  # caller falls back
    xb = x.astype(ml_dtypes.bfloat16)
    pad_to = int(bounds[:-1].max() + f_rows)
    if pad_to > n:
        xb = np.concatenate(
            [xb, np.zeros((pad_to - n, D), ml_dtypes.bfloat16)], axis=0)
    in_maps = []
    for c in range(n_cores):
        o = int(bounds[c])
        xs = np.ascontiguousarray(xb[o:o + f_rows])
        xsp = xs[perm]
        # [blk, p, k, r] = xsp[512 blk + r, 128k + p]: 4KB contiguous per
        # partition per block
        xtb = np.ascontiguousarray(
            xsp.reshape(nblk, 512, 4, 128).transpose(0, 3, 2, 1)
        ).astype(ml_dtypes.float8_e3m4)
        nb = min(f_rows, n - o) if n > o else 0
        br = np.full(f_rows, -1.0, dtype=np.float32)
        br[:nb] = batch[o:o + nb].astype(np.float32) - c * SEGS
        brp = br[perm]
        brel2d = np.ascontiguousarray(brp.reshape(tiles, 128).T)
        in_maps.append({
            "xr": xs, "xtb": xtb, "brel": brel2d, "w1": w1r, "w2": W2,
            "b1": b1, "b2": b2a, "cst": cst, "w18": w18,
        })
    return in_maps


def _numpy_fallback(x, batch, W1, b1, W2, b2):
    x = np.asarray(x, dtype=np.float32)
    batch = np.asarray(batch).astype(np.int64)
    scores = np.tanh(x @ W1 + b1) @ W2 + b2
    scores = scores - scores.max()
    e = np.exp(scores)
    den = np.zeros((B, 1), np.float32)
    np.add.at(den, batch, e)
    w = e / (den[batch] + 1e-8)
    out = np.zeros((B, D), np.float32)
    np.add.at(out, batch, w * x)
    return out


_RUNNER = {}


def _make_runner(nc, n_cores):
    """Reusable jitted SPMD executable (no donation) so repeated kernel()
    calls skip NEFF/XLA recompilation."""
    import jax
    import concourse.mybir as mybir
    from jax.sharding import Mesh, PartitionSpec, NamedSharding
    from jax.experimental.shard_map import shard_map
    from concourse import bass2jax

    bass2jax.install_neuronx_cc_hook()
    partition_name = (nc.partition_id_tensor.name
                      if nc.partition_id_tensor else None)
    in_names, out_names, out_avals, zero_outs = [], [], [], []
    for alloc in nc.m.functions[0].allocations:
        if not isinstance(alloc, mybir.MemoryLocationSet):
            continue
        name = alloc.memorylocations[0].name
        if alloc.kind == "ExternalInput":
            if name != partition_name:
                in_names.append(name)
        elif alloc.kind == "ExternalOutput":
            shape = tuple(alloc.tensor_shape)
            dtype = mybir.dt.np(alloc.dtype)
            out_names.append(name)
            out_avals.append(jax.core.ShapedArray(shape, dtype))
            zero_outs.append(np.zeros(shape, dtype))
    n_params = len(in_names)
    all_in_names = list(in_names) + list(out_names)
    if partition_name is not None:
        all_in_names.append(partition_name)

    def _body(*args):
        operands = list(args)
        if partition_name is not None:
            operands.append(bass2jax.partition_id_tensor())
        outs = bass2jax._bass_exec_p.bind(
            *operands,
            out_avals=tuple(out_avals),
            in_names=tuple(all_in_names),
            out_names=tuple(out_names),
            lowering_input_output_aliases=(),
            sim_require_finite=True,
            sim_require_nnan=True,
            nc=nc,
        )
        return tuple(outs)

    devices = jax.devices()[:n_cores]
    mesh = Mesh(np.asarray(devices), ("core",))
    nspec = n_params + len(out_names)
    fn = jax.jit(
        shard_map(_body, mesh=mesh,
                  in_specs=(PartitionSpec("core"),) * nspec,
                  out_specs=(PartitionSpec("core"),) * len(out_names),
                  check_rep=False),
        keep_unused=True,
    )
    sharding = NamedSharding(mesh, PartitionSpec("core"))
    concat_zero = [
        np.zeros((n_cores * z.shape[0], *z.shape[1:]), z.dtype) for z in zero_outs
    ]
    zero_dev = [jax.device_put(a, sharding) for a in concat_zero]
    return dict(fn=fn, in_names=in_names, out_names=out_names,
                out_avals=out_avals, zero_dev=zero_dev, sharding=sharding)


def _run_fast(nc, in_maps, n_cores):
    import jax
    if "r" not in _RUNNER:
        _RUNNER["r"] = _make_runner(nc, n_cores)
    r = _RUNNER["r"]
    concat_in = [
        np.concatenate([np.asarray(in_maps[c][name]) for c in range(n_cores)],
                       axis=0)
        for name in r["in_names"]
    ]
    dev_in = [jax.device_put(a, r["sharding"]) for a in concat_in]
    outs = r["fn"](*dev_in, *r["zero_dev"])
    jax.block_until_ready(outs)
    return [
        {name: np.asarray(outs[i]).reshape(n_cores, *r["out_avals"][i].shape)[c]
         for i, name in enumerate(r["out_names"])}
        for c in range(n_cores)
    ]


def kernel(x, batch, W1, b1, W2, b2):
    x = np.asarray(x)
    batch = np.asarray(batch)
    if (x.shape != (262144, D) or batch.shape != (262144,)
            or np.asarray(W1).shape != (D, H)):
        return _numpy_fallback(x, batch, W1, b1, W2, b2)
    if np.any(batch[:-1] > batch[1:]):
        return _numpy_fallback(x, batch, W1, b1, W2, b2)
    in_maps = make_in_maps(x, batch, W1, b1, W2, b2)
    if in_maps is None:
        return _numpy_fallback(x, batch, W1, b1, W2, b2)
    nc = get_nc()
    try:
        res = _run_fast(nc, in_maps, N_CORES)
        return np.concatenate([res[c]["out"] for c in range(N_CORES)], axis=0)
    except Exception:
        from concourse.bass_utils import run_bass_kernel_spmd
        res = run_bass_kernel_spmd(nc, in_maps, list(range(N_CORES)))
        return np.concatenate(
            [res.results[c]["out"] for c in range(N_CORES)], axis=0)


if __name__ == "__main__":
    pass


# revision 59
# speedup vs baseline: 1.0092x; 1.0092x over previous
"""AttentionPool (segment softmax-pool) Trainium2 kernel, 8 NeuronCores.

Math (reference):
    s = tanh(x @ W1 + b1) @ W2 + b2        # [N,1] scores
    e = exp(s - max(s))                    # global max shift
    out[b] = sum_{i in seg b} e_i x_i / (sum_{i in seg b} e_i + 1e-8)

Key identity: the global max shift cancels in the ratio (up to the
negligible 1e-8 term; |s| <= ||W2||_1 ~ 11 so exp never overflows), so we
compute e = exp(s) directly.  Every row's contribution is then local, and
with batch ids sorted, segments are contiguous runs.  Core c owns segments
[128c, 128(c+1)) and processes a fixed window of F rows starting at the
first row of segment 128c.  Rows of other cores' segments inside the
window self-mask: their relative id falls outside [0,128) so the one-hot
compare produces zero columns.

Numerics: pooling path in bf16; the shipped-transpose score path runs at
fp8 e3m4 (x and W1*32, compensated via tanh's input scale) - device-
measured end-to-end max rel err 1.60e-2 vs the 2e-2 gate (deterministic
for the fixed harness input).  e4m3 anywhere fails the gate (x-pool
3.9e-2, score-path 2.7e-2 even with weight prescaling); x-pool must
stay 2-byte.
The host ships TWO layouts of the window: row-major xr [F, D] bf16
(pooling matmul moving operand, DMA'd one block per DMA with 4
rows/partition so every partition is one contiguous 4KB descriptor; the
implied row permutation is folded into brel/xtb host-side) and
block-contiguous pre-transposed xtb [blk, 128, 4, 512] fp8e3 ([blk, p,
k, r] = x[512 blk + r, 128k + p], the W1 matmul moving operand, 2KB
/partition descriptors).  Scheduled blocks skip the xtb DMA and instead
PE-transpose the row-major tiles (PSUM bf16 -> one DVE copy to SBUF),
balancing the DMA engines against the PE.

    per 128-row tile on device (bf16 matmuls, 1 col/cycle @2.4GHz):
      u    = W1_k.T @ xts_k  (accum over k)            # [128h, 512r]
      th   = tanh(u + b1)  (bf16)                      # ACT
      s    = th.T @ W2 ; e = exp(s + b2)  (bf16)       # PE + ACT
      A    = (iota == brel) * e  (bf16)                # DVE
      num += A.T @ xr ; den += A.T @ ones_col          # PSUM f32 accum
    out = num * 1/(den + 1e-8), one [128,512] slab per core; host concat.

e is scalar-departitioned once per GB-block group via a DRAM bounce with
contiguous descriptors ([1, gn] -> [gt, 128]) plus a tiny PE transpose
to [128, gt] (the AP balancer cannot split partition 0 into 128
partitions in one hop, and a direct strided departition DMA costs 2048
4-byte descriptors ~ 5-9us of sequencer time per group).

Engine-queue scheduling: instructions execute in emission order per
engine, and the PE p-state model runs ~2x slower for the first 3us after
any idle gap, so emission is software-pipelined across blocks
(transposes for block b, W1 for b-1, score for b-2, pass2 num/den
matmuls lagged behind the bounce with A-builds one step ahead) so every
instruction's inputs are at least one block old when the engine reaches
it.  The dev/ship choice is time-scheduled (first 6 blocks dev while
the DMA queue warms up, last 8 ship so the PE-bound tail has no
transposes, 8 spread through the middle).  TimelineSim: 175.1us vs the
299.2us baseline (harness-measured 319.6us, sim tracked it within 7%).
"""

import os
import sys

for _p in ("/opt/trn_rl_repo",):
    if os.path.isdir(_p) and _p not in sys.path:
        sys.path.append(_p)

import numpy as np
import ml_dtypes

N_CORES = 8
B = 1024
SEGS = B // N_CORES          # 128 segments owned per core
D = 512
H = 128
F = 33792                    # fixed per-core row window (264 tiles of 128)
TILES = F // 128
DEV_NUM, DEV_DEN = 4, 9      # fraction of blocks transposed on-device


def build_nc(tiles=TILES, repeats=1, bufs=None, gb=8,
             dev_num=DEV_NUM, dev_den=DEV_DEN, ship_lead=2, oct_lead=2,
             lag=2, prefetch_at_end=False, dev_spread=False, tail_gb=4,
             ship_alt=False, x_alt=False,
             dev_sched=(6, 14, 12)):
    """Build the per-core Bass program. dev_num/dev_den: fraction of 4-tile
    blocks whose transposed layout is built on-device (PE transpose) rather
    than DMA'd from the host-shipped xtb. repeats>1 re-emits the whole
    computation for delta-timing."""
    bufs = {**dict(x=14, xts=6, th=5, A=8, es=3, xtp=2, u=2, e=2),
            **(bufs or {})}
    import concourse.bacc as bacc
    import concourse.mybir as mybir
    import concourse.tile as tile

    F32 = mybir.dt.float32
    F32R = mybir.dt.float32r
    BF16 = mybir.dt.bfloat16
    AF = mybir.ActivationFunctionType
    OP = mybir.AluOpType

    f_rows = tiles * 128
    assert tiles % 4 == 0
    nblk = tiles // 4

    nc = bacc.Bacc(None, target_bir_lowering=False)
    cst_d = nc.dram_tensor("cst", (128, 257), BF16, kind="ExternalInput")
    xr_d = nc.dram_tensor("xr", (f_rows, D), BF16, kind="ExternalInput")
    FP8E3 = __import__('concourse.mybir', fromlist=['dt']).dt.float8e3
    xtb_d = nc.dram_tensor("xtb", (nblk, 128, 4, 512), FP8E3,
                           kind="ExternalInput")
    w18_d = nc.dram_tensor("w18", (128, 4, H), FP8E3, kind="ExternalInput")
    brel_d = nc.dram_tensor("brel", (128, tiles), F32, kind="ExternalInput")
    w1_d = nc.dram_tensor("w1", (128, 4, H), BF16, kind="ExternalInput")
    w2_d = nc.dram_tensor("w2", (H, 2), BF16, kind="ExternalInput")
    b1_d = nc.dram_tensor("b1", (H, 1), F32, kind="ExternalInput")
    b2_d = nc.dram_tensor("b2", (128, 1), F32, kind="ExternalInput")
    out_d = nc.dram_tensor("out", (SEGS, D), F32, kind="ExternalOutput")

    def is_dev_block(blk):
        # dev_num of every dev_den blocks are transposed on-device
        if dev_sched is not None:
            if len(dev_sched) == 4:
                front, total, tail_s, lead_ship = dev_sched
            else:
                front, total, tail_s = dev_sched
                lead_ship = 0
            if blk < lead_ship:
                return False
            if blk < front + lead_ship:
                return True
            blk = blk - lead_ship
            last = nblk - tail_s - lead_ship
            if blk >= last:
                return False
            mid = last - front
            need = total - front
            r = blk - front
            return (r * need) // mid != ((r + 1) * need) // mid
        if dev_spread:
            r = blk % dev_den
            return (r * dev_num) // dev_den != ((r + 1) * dev_num) // dev_den
        return (blk % dev_den) < dev_num

    with tile.TileContext(nc) as tc:
        with (
            tc.tile_pool(name="const", bufs=1) as cpool,
            tc.tile_pool(name="xin", bufs=bufs["x"]) as xpool,
            tc.tile_pool(name="xts", bufs=bufs["xts"]) as xtspool,
            tc.tile_pool(name="xts8", bufs=bufs["xts"]) as xts8pool,
            tc.tile_pool(name="th", bufs=bufs["th"]) as thpool,
            tc.tile_pool(name="abuild", bufs=bufs["A"]) as apool,
            tc.tile_pool(name="esb", bufs=bufs["es"]) as espool,
            tc.tile_pool(name="e4", bufs=bufs["es"]) as e4pool,
            tc.tile_pool(name="edram", bufs=bufs["es"], space="DRAM") as dramp,
            tc.tile_pool(name="fin", bufs=1) as fpool,
            tc.tile_pool(name="ps_xt", bufs=bufs["xtp"], space="PSUM") as xtpsum,
            tc.tile_pool(name="ps_u", bufs=bufs["u"], space="PSUM") as upsum,
            tc.tile_pool(name="ps_e", bufs=bufs["e"], space="PSUM") as epsum,
            tc.tile_pool(name="ps_num", bufs=1, space="PSUM") as numpsum,
            tc.tile_pool(name="ps_den", bufs=1, space="PSUM") as denpsum,
        ):
            # ---- constants ----
            w1b = cpool.tile([128, 4, H], BF16)
            nc.sync.dma_start(w1b[:], w1_d[:])
            w18 = cpool.tile([128, 4, H], FP8E3)
            nc.sync.dma_start(w18[:], w18_d[:])
            w2r = cpool.tile([H, 2], BF16)
            nc.scalar.dma_start(w2r[:], w2_d[:])
            b1s = cpool.tile([H, 1], F32)
            nc.scalar.dma_start(b1s[:], b1_d[:])
            b2s = cpool.tile([128, 1], F32)
            nc.scalar.dma_start(b2s[:], b2_d[:])
            brel = cpool.tile([128, tiles], F32)
            brel_loaded = [False]

            # host-shipped constants: [identity | iota-row | ones] bf16
            cst = cpool.tile([128, 257], BF16)
            nc.sync.dma_start(cst[:], cst_d[:])
            identb = cst[:, 0:128]     # identity for PE transposes
            iifb = cst[:, 128:256]     # iifb[p, j] = j, for the A build
            onesc = cst[:, 256:257]

            num = numpsum.tile([SEGS, D], F32)
            denc = denpsum.tile([SEGS, 1], F32)

            GB = gb  # blocks per departition group
            LAG = lag  # extra blocks between a group's bounce and its pass2
            nblk_b = tiles // 4
            copy_alt = [0]
            for rep in range(repeats):
                x8s = {}             # octet index -> tile (kept for pass2)
                xtss = {}            # block -> xts tile (kept one block)
                ths = {}             # block -> th tile (kept one block)
                ess = {}             # group -> es1 tile
                pend_blocks = []     # bases of blocks awaiting pass2 grouping
                grp_start = [0]      # first block of the current group
                p2q = []             # FIFO of pending pass2 groups

                def emit_abuild_block(grp):
                    """A-builds (DVE) for one block of a group, one pipeline
                    step ahead of its num/den matmuls.  First call also emits
                    the tiny e-departition transpose."""
                    if grp["e4"] is None:
                        gt = grp["gt"]
                        e4p = epsum.tile([128, GB * 4], BF16, tag="e")
                        nc.tensor.transpose(
                            e4p[:, 0:gt], grp["em"][0:gt, :],
                            identb[0:gt, 0:gt],
                        )
                        e4 = e4pool.tile([128, GB * 4], F32, tag="e4")
                        nc.vector.tensor_copy(e4[:, 0:gt], e4p[:, 0:gt])
                        grp["e4"] = e4
                    base2 = grp["blocks"].pop(0)
                    A4 = apool.tile([128, 4, SEGS], BF16, tag="A")
                    for q2 in range(4):
                        tt = base2 + q2
                        ecol = tt - 4 * grp["start"]
                        nc.vector.tensor_scalar(
                            A4[:, q2, :], iifb[:], brel[:, tt:tt + 1],
                            grp["e4"][:, ecol:ecol + 1],
                            op0=OP.is_equal, op1=OP.mult,
                        )
                    return (base2, A4)

                def emit_pass2_block(built):
                    """num/den matmuls for a block whose A was built one
                    pipeline step earlier."""
                    base2, A4 = built
                    for q2 in range(4):
                        tt = base2 + q2
                        nc.tensor.matmul(
                            num[:], A4[:, q2, :],
                            x8s[tt // 4][:, tt % 4, :],
                            start=(tt == 0), stop=(tt == tiles - 1),
                            skip_group_check=True,
                        )
                        # den rides the same stationary A: one extra column
                        nc.tensor.matmul(
                            denc[:], A4[:, q2, :], onesc[:],
                            start=(tt == 0), stop=(tt == tiles - 1),
                            skip_group_check=True,
                        )
                    done = [o for o in x8s if (o + 1) * 4 <= base2 + 4]
                    for o in done:
                        del x8s[o]

                built_q = []   # A-built pass2 blocks awaiting matmuls
                noct = tiles // 8

                def emit_prefetch(bi):
                    # x8 octets oct_lead octets ahead, ship DMAs ship_lead
                    # blocks ahead of their W1 stage
                    for blk_i in (list(range(2 * oct_lead + 1))
                                  if bi == 0 else [bi + 2 * oct_lead]):
                        if blk_i < nblk_b:
                            t0 = blk_i * 4
                            # one DMA per block; partition p holds 4 window
                            # rows 512g+4p..4p+3 as one contiguous 4KB
                            # descriptor (host permutes brel/xtb to match)
                            x4 = xpool.tile([128, 4, D], BF16, tag="x")
                            xq_eng = (nc.scalar if (x_alt and blk_i % 2 == 1)
                                      else nc.sync)
                            xq_eng.dma_start(
                                x4[:],
                                xr_d[t0 * 128:(t0 + 4) * 128, :].rearrange(
                                    "(p a) d -> p a d", p=128),
                            )
                            x8s[blk_i] = x4
                    for lead_bi in (list(range(ship_lead + 1)) if bi == 0
                                    else [bi + ship_lead]):
                        if lead_bi < nblk_b:
                            if is_dev_block(lead_bi):
                                xts = xtspool.tile([128, 4, D], BF16,
                                                   tag="xts", name="xts")
                            else:
                                xts = xts8pool.tile([128, 4, D], FP8E3,
                                                    tag="xts8", name="xts")
                                shipq = (nc.sync if (ship_alt and
                                                     lead_bi % 2 == 0)
                                         else nc.scalar)
                                shipq.dma_start(xts[:], xtb_d[lead_bi])
                            xtss[lead_bi] = xts

                if prefetch_at_end:
                    emit_prefetch(0)
                    if not brel_loaded[0]:
                        nc.scalar.dma_start(brel[:], brel_d[:])
                        brel_loaded[0] = True
                for bi in range(nblk_b + 3):
                    if not prefetch_at_end:
                        emit_prefetch(bi)
                    if not brel_loaded[0]:
                        nc.scalar.dma_start(brel[:], brel_d[:])
                        brel_loaded[0] = True
                    if bi < nblk_b:
                        for q in range(4):
                            t = 4 * bi + q
                            if is_dev_block(bi):
                                xt = x8s[t // 4][:, t % 4, :]
                                xq = xtpsum.tile([128, D], BF16, tag="xtp")
                                for k in range(4):
                                    nc.tensor.transpose(
                                        xq[:, k * 128:(k + 1) * 128],
                                        xt[:, k * 128:(k + 1) * 128],
                                        identb[:],
                                    )
                                dst = xtss[bi][:, :, q * 128:(q + 1) * 128]
                                src = xq[:].rearrange("p (k r) -> p k r", k=4)
                                nc.vector.tensor_copy(dst, src)
                                copy_alt[0] += 1

                    # ---- stage 1 (block bi-1): W1 matmuls + tanh
                    b1i = bi - 1
                    if 0 <= b1i < nblk_b:
                        nb = 4 * 128
                        xts1 = xtss.pop(b1i)
                        dev1 = is_dev_block(b1i)
                        w1use = w1b if dev1 else w18
                        u = upsum.tile([H, 4 * 128], F32, tag="u")
                        for k in range(4):
                            nc.tensor.matmul(
                                u[:, 0:nb],
                                w1use[:, k, :],
                                xts1[:, k, 0:nb],
                                start=(k == 0),
                                stop=(k == 3),
                            )
                        th = thpool.tile([H, 4 * 128], BF16, tag="th")
                        # ship blocks run the MLP at fp8e3 with W1
                        # pre-scaled x32; tanh's input scale compensates
                        nc.scalar.activation(
                            th[:, 0:nb], u[:, 0:nb],
                            AF.Tanh, bias=b1s[:],
                            scale=(1.0 if dev1 else 1.0 / 32.0),
                        )
                        ths[b1i] = th

                    # ---- stage 2 (block bi-2): score matmul + exp + bounce
                    b2i = bi - 2
                    if 0 <= b2i < nblk_b:
                        nb = 4 * 128
                        th2 = ths.pop(b2i)
                        ep2 = epsum.tile([2, 4 * 128], F32, tag="e")
                        nc.tensor.matmul(
                            ep2[:, 0:nb], w2r[:], th2[:, 0:nb],
                            start=True, stop=True, skip_group_check=True,
                        )
                        g = b2i - grp_start[0]      # slot within group
                        if g == 0:
                            es1 = espool.tile([1, GB * 512], BF16,
                                              tag="es", name="es1")
                            ess[0] = es1
                        es1 = ess[0]
                        nc.scalar.activation(
                            es1[0:1, g * 512:g * 512 + nb], ep2[0:1, 0:nb],
                            AF.Exp, bias=b2s[0:1, :], scale=1.0,
                        )
                        pend_blocks.append(4 * b2i)
                        rem_after = nblk_b - 1 - b2i
                        gb_cur = GB if (tail_gb == 0
                                        or rem_after >= GB + 2) else tail_gb
                        last_of_group = (g == gb_cur - 1) or (b2i == nblk_b - 1)
                        if last_of_group:
                            gs = grp_start[0]
                            grp_start[0] = b2i + 1
                            gn = g * 512 + nb    # valid scalars in group
                            gt = (gn + 127) // 128
                            # departition e: [1, gn] -> [gt, 128] via a DRAM
                            # bounce with contiguous descriptors; the tiny
                            # PE transpose to [128, gt] is emitted lagged in
                            # emit_pass2_block so PE never waits on it
                            ed = dramp.tile([GB * 512], BF16, tag="ed")
                            nc.scalar.dma_start(ed[0:gn], es1[0:1, 0:gn])
                            em = e4pool.tile([GB * 4, 128], BF16, tag="em")
                            nc.scalar.dma_start(
                                em[0:gt, :],
                                ed[0:gn].rearrange("(a u) -> a u", a=gt),
                            )
                            del ess[0]
                            p2q.append(dict(blocks=pend_blocks, em=em,
                                            gt=gt, e4=None, enq=bi, start=gs))
                            pend_blocks = []

                    # ---- lagged pass2, two sub-stages one step apart:
                    # num/den matmuls for the block A-built last step, then
                    # A-builds (DVE) for the next block
                    if built_q:
                        emit_pass2_block(built_q.pop(0))
                    if p2q and bi - p2q[0]["enq"] >= LAG:
                        built_q.append(emit_abuild_block(p2q[0]))
                        if not p2q[0]["blocks"]:
                            p2q.pop(0)
                    if prefetch_at_end:
                        emit_prefetch(bi + 1)
                # drain remaining pass2 groups
                while p2q or built_q:
                    if built_q:
                        emit_pass2_block(built_q.pop(0))
                    if p2q:
                        built_q.append(emit_abuild_block(p2q[0]))
                        if not p2q[0]["blocks"]:
                            p2q.pop(0)

                dsb = fpool.tile([SEGS, 1], F32)
                nc.vector.tensor_scalar(dsb[:], denc[:, 0:1],
                                        1e-8, None, op0=OP.add)
                rec = fpool.tile([SEGS, 1], F32)
                nc.vector.reciprocal(rec[:], dsb[:])
                osb = fpool.tile([SEGS, D], F32)
                nc.vector.tensor_scalar(osb[:], num[:], rec[:], None,
                                        op0=OP.mult)
                nc.sync.dma_start(out_d[:], osb[:])

    nc.compile()
    return nc


_NC_CACHE = {}


def get_nc(tiles=TILES):
    if tiles not in _NC_CACHE:
        _NC_CACHE[tiles] = build_nc(tiles)
    return _NC_CACHE[tiles]


def make_in_maps(x, batch, W1, b1, W2, b2, tiles=TILES, n_cores=N_CORES):
    """Host-side sharding: segment-aligned fixed windows + relative ids,
    both bf16 layouts of each window, pre-arranged W1.

    The device loads xr with 8 rows per partition per DMA (one contiguous
    8KB descriptor per partition): window position j = 1024g + 128a + p
    holds original window row 1024g + 8p + a.  brel and xtb are built in
    this permuted order so everything downstream stays consistent."""
    x = np.ascontiguousarray(np.asarray(x, dtype=np.float32))
    batch = np.asarray(batch).astype(np.int64)
    W1 = np.asarray(W1, dtype=np.float32)
    b1 = np.asarray(b1, dtype=np.float32).reshape(H, 1)
    W2 = np.ascontiguousarray(
        np.repeat(np.asarray(W2, dtype=np.float32).reshape(H, 1), 2, axis=1)
    ).astype(ml_dtypes.bfloat16)
    b2v = float(np.asarray(b2, dtype=np.float32).reshape(-1)[0])
    b2a = np.full((128, 1), b2v, dtype=np.float32)
    # w1 rearranged: [p, k, h] = W1[128k + p, h]; w18 = fp8e3 of W1*32
    w1p = np.ascontiguousarray(W1.reshape(4, 128, H).transpose(1, 0, 2))
    w1r = w1p.astype(ml_dtypes.bfloat16)
    w18 = (w1p * 32.0).astype(ml_dtypes.float8_e3m4)
    # device constants: [identity | iota-row | ones] bf16
    cst = np.zeros((128, 257), dtype=ml_dtypes.bfloat16)
    cst[:, 0:128] = np.eye(128, dtype=np.float32)
    cst[:, 128:256] = np.arange(128, dtype=np.float32)[None, :]
    cst[:, 256] = 1.0

    n = x.shape[0]
    f_rows = tiles * 128
    nblk = tiles // 4
    # perm[j] = original window row at permuted position j
    perm = (np.arange(f_rows).reshape(-1, 128, 4)
            .transpose(0, 2, 1).reshape(f_rows))
    bounds = np.searchsorted(batch, np.arange(0, n_cores + 1) * SEGS)
    owned = np.diff(bounds)
    if owned.max() > f_rows:
        return None  # caller falls back
    xb = x.astype(ml_dtypes.bfloat16)
    pad_to = int(bounds[:-1].max() + f_rows)
    if pad_to > n:
        xb = np.concatenate(
            [xb, np.zeros((pad_to - n, D), ml_dtypes.bfloat16)], axis=0)
    in_maps = []
    for c in range(n_cores):
        o = int(bounds[c])
        xs = np.ascontiguousarray(xb[o:o + f_rows])
        xsp = xs[perm]
        # [blk, p, k, r] = xsp[512 blk + r, 128k + p]: 4KB contiguous per
        # partition per block
        xtb = np.ascontiguousarray(
            xsp.reshape(nblk, 512, 4, 128).transpose(0, 3, 2, 1)
        ).astype(ml_dtypes.float8_e3m4)
        nb = min(f_rows, n - o) if n > o else 0
        br = np.full(f_rows, -1.0, dtype=np.float32)
        br[:nb] = batch[o:o + nb].astype(np.float32) - c * SEGS
        brp = br[perm]
        brel2d = np.ascontiguousarray(brp.reshape(tiles, 128).T)
        in_maps.append({
            "xr": xs, "xtb": xtb, "brel": brel2d, "w1": w1r, "w2": W2,
            "b1": b1, "b2": b2a, "cst": cst, "w18": w18,
        })
    return in_maps


def _numpy_fallback(x, batch, W1, b1, W2, b2):
    x = np.asarray(x, dtype=np.float32)
    batch = np.asarray(batch).astype(np.int64)
    scores = np.tanh(x @ W1 + b1) @ W2 + b2
    scores = scores - scores.max()
    e = np.exp(scores)
    den = np.zeros((B, 1), np.float32)
    np.add.at(den, batch, e)
    w = e / (den[batch] + 1e-8)
    out = np.zeros((B, D), np.float32)
    np.add.at(out, batch, w * x)
    return out


_RUNNER = {}


def _make_runner(nc, n_cores):
    """Reusable jitted SPMD executable (no donation) so repeated kernel()
    calls skip NEFF/XLA recompilation."""
    import jax
    import concourse.mybir as mybir
    from jax.sharding import Mesh, PartitionSpec, NamedSharding
    from jax.experimental.shard_map import shard_map
    from concourse import bass2jax

    bass2jax.install_neuronx_cc_hook()
    partition_name = (nc.partition_id_tensor.name
                      if nc.partition_id_tensor else None)
    in_names, out_names, out_avals, zero_outs = [], [], [], []
    for alloc in nc.m.functions[0].allocations:
        if not isinstance(alloc, mybir.MemoryLocationSet):
            continue
        name = alloc.memorylocations[0].name
        if alloc.kind == "ExternalInput":
            if name != partition_name:
                in_names.append(name)
        elif alloc.kind == "ExternalOutput":
            shape = tuple(alloc.tensor_shape)
            dtype = mybir.dt.np(alloc.dtype)
            out_names.append(name)
            out_avals.append(jax.core.ShapedArray(shape, dtype))
            zero_outs.append(np.zeros(shape, dtype))
    n_params = len(in_names)
    all_in_names = list(in_names) + list(out_names)
    if partition_name is not None:
        all_in_names.append(partition_name)

    def _body(*args):
        operands = list(args)
        if partition_name is not None:
            operands.append(bass2jax.partition_id_tensor())
        outs = bass2jax._bass_exec_p.bind(
            *operands,
            out_avals=tuple(out_avals),
            in_names=tuple(all_in_names),
            out_names=tuple(out_names),
            lowering_input_output_aliases=(),
            sim_require_finite=True,
            sim_require_nnan=True,
            nc=nc,
        )
        return tuple(outs)

    devices = jax.devices()[:n_cores]
    mesh = Mesh(np.asarray(devices), ("core",))
    nspec = n_params + len(out_names)
    fn = jax.jit(
        shard_map(_body, mesh=mesh,
                  in_specs=(PartitionSpec("core"),) * nspec,
                  out_specs=(PartitionSpec("core"),) * len(out_names),
                  check_rep=False),
        keep_unused=True,
    )
    sharding = NamedSharding(mesh, PartitionSpec("core"))
    concat_zero = [
        np.zeros((n_cores * z.shape[0], *z.shape[1:]), z.dtype) for z in zero_outs
    ]
    zero_dev = [jax.device_put(a, sharding) for a in concat_zero]
    return dict(fn=fn, in_names=in_names, out_names=out_names,
                out_avals=out_avals, zero_dev=zero_dev, sharding=sharding)


def _run_fast(nc, in_maps, n_cores):
    import jax
    if "r" not in _RUNNER:
        _RUNNER["r"] = _make_runner(nc, n_cores)
    r = _RUNNER["r"]
    concat_in = [
        np.concatenate([np.asarray(in_maps[c][name]) for c in range(n_cores)],
                       axis=0)
        for name in r["in_names"]
    ]
    dev_in = [jax.device_put(a, r["sharding"]) for a in concat_in]
    outs = r["fn"](*dev_in, *r["zero_dev"])
    jax.block_until_ready(outs)
    return [
        {name: np.asarray(outs[i]).reshape(n_cores, *r["out_avals"][i].shape)[c]
         for i, name in enumerate(r["out_names"])}
        for c in range(n_cores)
    ]


def kernel(x, batch, W1, b1, W2, b2):
    x = np.asarray(x)
    batch = np.asarray(batch)
    if (x.shape != (262144, D) or batch.shape != (262144,)
            or np.asarray(W1).shape != (D, H)):
        return _numpy_fallback(x, batch, W1, b1, W2, b2)
    if np.any(batch[:-1] > batch[1:]):
        return _numpy_fallback(x, batch, W1, b1, W2, b2)
    in_maps = make_in_maps(x, batch, W1, b1, W2, b2)
    if in_maps is None:
        return _numpy_fallback(x, batch, W1, b1, W2, b2)
    nc = get_nc()
    try:
        res = _run_fast(nc, in_maps, N_CORES)
        return np.concatenate([res[c]["out"] for c in range(N_CORES)], axis=0)
    except Exception:
        from concourse.bass_utils import run_bass_kernel_spmd
        res = run_bass_kernel_spmd(nc, in_maps, list(range(N_CORES)))
        return np.concatenate(
            [res.results[c]["out"] for c in range(N_CORES)], axis=0)


if __name__ == "__main__":
    pass


# revision 61
# speedup vs baseline: 1.0258x; 1.0165x over previous
"""AttentionPool (segment softmax-pool) Trainium2 kernel, 8 NeuronCores.

Math (reference):
    s = tanh(x @ W1 + b1) @ W2 + b2        # [N,1] scores
    e = exp(s - max(s))                    # global max shift
    out[b] = sum_{i in seg b} e_i x_i / (sum_{i in seg b} e_i + 1e-8)

Key identity: the global max shift cancels in the ratio (up to the
negligible 1e-8 term; |s| <= ||W2||_1 ~ 11 so exp never overflows), so we
compute e = exp(s) directly.  Every row's contribution is then local, and
with batch ids sorted, segments are contiguous runs.  Core c owns segments
[128c, 128(c+1)) and processes a fixed window of F rows starting at the
first row of segment 128c.  Rows of other cores' segments inside the
window self-mask: their relative id falls outside [0,128) so the one-hot
compare produces zero columns.

Numerics: pooling path in bf16; the shipped-transpose score path runs at
fp8 e3m4 (x and W1*32, compensated via tanh's input scale) - device-
measured end-to-end max rel err 1.60e-2 vs the 2e-2 gate (deterministic
for the fixed harness input).  e4m3 anywhere fails the gate (x-pool
3.9e-2, score-path 2.7e-2 even with weight prescaling); x-pool must
stay 2-byte.
The host ships TWO layouts of the window: row-major xr [F, D] bf16
(pooling matmul moving operand, DMA'd one block per DMA with 4
rows/partition so every partition is one contiguous 4KB descriptor; the
implied row permutation is folded into brel/xtb host-side) and
block-contiguous pre-transposed xtb [blk, 128, 4, 512] fp8e3 ([blk, p,
k, r] = x[512 blk + r, 128k + p], the W1 matmul moving operand, 2KB
/partition descriptors).  Scheduled blocks skip the xtb DMA and instead
PE-transpose the row-major tiles (PSUM bf16 -> one DVE copy to SBUF),
balancing the DMA engines against the PE.

    per 128-row tile on device (bf16 matmuls, 1 col/cycle @2.4GHz):
      u    = W1_k.T @ xts_k  (accum over k)            # [128h, 512r]
      th   = tanh(u + b1)  (bf16)                      # ACT
      s    = th.T @ W2 ; e = exp(s + b2)  (bf16)       # PE + ACT
      A    = (iota == brel) * e  (bf16)                # DVE
      num += A.T @ xr ; den += A.T @ ones_col          # PSUM f32 accum
    out = num * 1/(den + 1e-8), one [128,512] slab per core; host concat.

e is scalar-departitioned once per GB-block group via a DRAM bounce with
contiguous descriptors ([1, gn] -> [gt, 128]) plus a tiny PE transpose
to [128, gt] (the AP balancer cannot split partition 0 into 128
partitions in one hop, and a direct strided departition DMA costs 2048
4-byte descriptors ~ 5-9us of sequencer time per group).

Engine-queue scheduling: instructions execute in emission order per
engine, and the PE p-state model runs ~2x slower for the first 3us after
any idle gap, so emission is software-pipelined across blocks
(transposes for block b, W1 for b-1, score for b-2, pass2 num/den
matmuls lagged behind the bounce with A-builds one step ahead) so every
instruction's inputs are at least one block old when the engine reaches
it.  The dev/ship choice is time-scheduled (first 6 blocks dev while
the DMA queue warms up, last 8 ship so the PE-bound tail has no
transposes, 8 spread through the middle).  TimelineSim: 170.7us vs the
299.2us baseline (harness-measured 319.6us, sim tracked it within 7%).
"""

import os
import sys

for _p in ("/opt/trn_rl_repo",):
    if os.path.isdir(_p) and _p not in sys.path:
        sys.path.append(_p)

import numpy as np
import ml_dtypes

N_CORES = 8
B = 1024
SEGS = B // N_CORES          # 128 segments owned per core
D = 512
H = 128
F = 33792                    # fixed per-core row window (264 tiles of 128)
TILES = F // 128
DEV_NUM, DEV_DEN = 4, 9      # fraction of blocks transposed on-device


def build_nc(tiles=TILES, repeats=1, bufs=None, gb=8,
             dev_num=DEV_NUM, dev_den=DEV_DEN, ship_lead=2, oct_lead=2,
             lag=2, prefetch_at_end=False, dev_spread=False, tail_gb=4,
             ship_alt=False, x_alt=False,
             dev_sched=(6, 14, 10)):
    """Build the per-core Bass program. dev_num/dev_den: fraction of 4-tile
    blocks whose transposed layout is built on-device (PE transpose) rather
    than DMA'd from the host-shipped xtb. repeats>1 re-emits the whole
    computation for delta-timing."""
    bufs = {**dict(x=14, xts=6, th=5, A=8, es=3, xtp=2, u=2, e=2),
            **(bufs or {})}
    import concourse.bacc as bacc
    import concourse.mybir as mybir
    import concourse.tile as tile

    F32 = mybir.dt.float32
    F32R = mybir.dt.float32r
    BF16 = mybir.dt.bfloat16
    AF = mybir.ActivationFunctionType
    OP = mybir.AluOpType

    f_rows = tiles * 128
    assert tiles % 4 == 0
    nblk = tiles // 4

    nc = bacc.Bacc(None, target_bir_lowering=False)
    cst_d = nc.dram_tensor("cst", (128, 257), BF16, kind="ExternalInput")
    xr_d = nc.dram_tensor("xr", (f_rows, D), BF16, kind="ExternalInput")
    FP8E3 = __import__('concourse.mybir', fromlist=['dt']).dt.float8e3
    xtb_d = nc.dram_tensor("xtb", (nblk, 128, 4, 512), FP8E3,
                           kind="ExternalInput")
    w18_d = nc.dram_tensor("w18", (128, 4, H), FP8E3, kind="ExternalInput")
    brel_d = nc.dram_tensor("brel", (128, tiles), F32, kind="ExternalInput")
    w1_d = nc.dram_tensor("w1", (128, 4, H), BF16, kind="ExternalInput")
    w2_d = nc.dram_tensor("w2", (H, 2), BF16, kind="ExternalInput")
    b1_d = nc.dram_tensor("b1", (H, 1), F32, kind="ExternalInput")
    b2_d = nc.dram_tensor("b2", (128, 1), F32, kind="ExternalInput")
    out_d = nc.dram_tensor("out", (SEGS, D), F32, kind="ExternalOutput")

    def is_dev_block(blk):
        # dev_num of every dev_den blocks are transposed on-device
        if dev_sched is not None:
            if len(dev_sched) == 4:
                front, total, tail_s, lead_ship = dev_sched
            else:
                front, total, tail_s = dev_sched
                lead_ship = 0
            if blk < lead_ship:
                return False
            if blk < front + lead_ship:
                return True
            blk = blk - lead_ship
            last = nblk - tail_s - lead_ship
            if blk >= last:
                return False
            mid = last - front
            need = total - front
            r = blk - front
            return (r * need) // mid != ((r + 1) * need) // mid
        if dev_spread:
            r = blk % dev_den
            return (r * dev_num) // dev_den != ((r + 1) * dev_num) // dev_den
        return (blk % dev_den) < dev_num

    with tile.TileContext(nc) as tc:
        with (
            tc.tile_pool(name="const", bufs=1) as cpool,
            tc.tile_pool(name="xin", bufs=bufs["x"]) as xpool,
            tc.tile_pool(name="xts", bufs=bufs["xts"]) as xtspool,
            tc.tile_pool(name="xts8", bufs=bufs["xts"]) as xts8pool,
            tc.tile_pool(name="th", bufs=bufs["th"]) as thpool,
            tc.tile_pool(name="abuild", bufs=bufs["A"]) as apool,
            tc.tile_pool(name="esb", bufs=bufs["es"]) as espool,
            tc.tile_pool(name="e4", bufs=bufs["es"]) as e4pool,
            tc.tile_pool(name="edram", bufs=bufs["es"], space="DRAM") as dramp,
            tc.tile_pool(name="fin", bufs=1) as fpool,
            tc.tile_pool(name="ps_xt", bufs=bufs["xtp"], space="PSUM") as xtpsum,
            tc.tile_pool(name="ps_u", bufs=bufs["u"], space="PSUM") as upsum,
            tc.tile_pool(name="ps_e", bufs=bufs["e"], space="PSUM") as epsum,
            tc.tile_pool(name="ps_num", bufs=1, space="PSUM") as numpsum,
            tc.tile_pool(name="ps_den", bufs=1, space="PSUM") as denpsum,
        ):
            # ---- constants ----
            w1b = cpool.tile([128, 4, H], BF16)
            nc.sync.dma_start(w1b[:], w1_d[:])
            w18 = cpool.tile([128, 4, H], FP8E3)
            nc.sync.dma_start(w18[:], w18_d[:])
            w2r = cpool.tile([H, 2], BF16)
            nc.scalar.dma_start(w2r[:], w2_d[:])
            b1s = cpool.tile([H, 1], F32)
            nc.scalar.dma_start(b1s[:], b1_d[:])
            b2s = cpool.tile([128, 1], F32)
            nc.scalar.dma_start(b2s[:], b2_d[:])
            brel = cpool.tile([128, tiles], F32)
            brel_loaded = [False]

            # host-shipped constants: [identity | iota-row | ones] bf16
            cst = cpool.tile([128, 257], BF16)
            nc.sync.dma_start(cst[:], cst_d[:])
            identb = cst[:, 0:128]     # identity for PE transposes
            iifb = cst[:, 128:256]     # iifb[p, j] = j, for the A build
            onesc = cst[:, 256:257]

            num = numpsum.tile([SEGS, D], F32)
            denc = denpsum.tile([SEGS, 1], F32)

            GB = gb  # blocks per departition group
            LAG = lag  # extra blocks between a group's bounce and its pass2
            nblk_b = tiles // 4
            copy_alt = [0]
            for rep in range(repeats):
                x8s = {}             # octet index -> tile (kept for pass2)
                xtss = {}            # block -> xts tile (kept one block)
                ths = {}             # block -> th tile (kept one block)
                ess = {}             # group -> es1 tile
                pend_blocks = []     # bases of blocks awaiting pass2 grouping
                grp_start = [0]      # first block of the current group
                p2q = []             # FIFO of pending pass2 groups

                def emit_abuild_block(grp):
                    """A-builds (DVE) for one block of a group, one pipeline
                    step ahead of its num/den matmuls.  First call also emits
                    the tiny e-departition transpose."""
                    if grp["e4"] is None:
                        gt = grp["gt"]
                        e4p = epsum.tile([128, GB * 4], BF16, tag="e")
                        nc.tensor.transpose(
                            e4p[:, 0:gt], grp["em"][0:gt, :],
                            identb[0:gt, 0:gt],
                        )
                        e4 = e4pool.tile([128, GB * 4], F32, tag="e4")
                        nc.vector.tensor_copy(e4[:, 0:gt], e4p[:, 0:gt])
                        grp["e4"] = e4
                    base2 = grp["blocks"].pop(0)
                    A4 = apool.tile([128, 4, SEGS], BF16, tag="A")
                    for q2 in range(4):
                        tt = base2 + q2
                        ecol = tt - 4 * grp["start"]
                        nc.vector.tensor_scalar(
                            A4[:, q2, :], iifb[:], brel[:, tt:tt + 1],
                            grp["e4"][:, ecol:ecol + 1],
                            op0=OP.is_equal, op1=OP.mult,
                        )
                    return (base2, A4)

                def emit_pass2_block(built):
                    """num/den matmuls for a block whose A was built one
                    pipeline step earlier."""
                    base2, A4 = built
                    for q2 in range(4):
                        tt = base2 + q2
                        nc.tensor.matmul(
                            num[:], A4[:, q2, :],
                            x8s[tt // 4][:, tt % 4, :],
                            start=(tt == 0), stop=(tt == tiles - 1),
                            skip_group_check=True,
                        )
                        # den rides the same stationary A: one extra column
                        nc.tensor.matmul(
                            denc[:], A4[:, q2, :], onesc[:],
                            start=(tt == 0), stop=(tt == tiles - 1),
                            skip_group_check=True,
                        )
                    done = [o for o in x8s if (o + 1) * 4 <= base2 + 4]
                    for o in done:
                        del x8s[o]

                built_q = []   # A-built pass2 blocks awaiting matmuls
                noct = tiles // 8

                def emit_prefetch(bi):
                    # x8 octets oct_lead octets ahead, ship DMAs ship_lead
                    # blocks ahead of their W1 stage
                    for blk_i in (list(range(2 * oct_lead + 1))
                                  if bi == 0 else [bi + 2 * oct_lead]):
                        if blk_i < nblk_b:
                            t0 = blk_i * 4
                            # one DMA per block; partition p holds 4 window
                            # rows 512g+4p..4p+3 as one contiguous 4KB
                            # descriptor (host permutes brel/xtb to match)
                            x4 = xpool.tile([128, 4, D], BF16, tag="x")
                            xq_eng = (nc.scalar if (x_alt and blk_i % 2 == 1)
                                      else nc.sync)
                            xq_eng.dma_start(
                                x4[:],
                                xr_d[t0 * 128:(t0 + 4) * 128, :].rearrange(
                                    "(p a) d -> p a d", p=128),
                            )
                            x8s[blk_i] = x4
                    for lead_bi in (list(range(ship_lead + 1)) if bi == 0
                                    else [bi + ship_lead]):
                        if lead_bi < nblk_b:
                            if is_dev_block(lead_bi):
                                xts = xtspool.tile([128, 4, D], BF16,
                                                   tag="xts", name="xts")
                            else:
                                xts = xts8pool.tile([128, 4, D], FP8E3,
                                                    tag="xts8", name="xts")
                                shipq = (nc.sync if (ship_alt and
                                                     lead_bi % 2 == 0)
                                         else nc.scalar)
                                shipq.dma_start(xts[:], xtb_d[lead_bi])
                            xtss[lead_bi] = xts

                if prefetch_at_end:
                    emit_prefetch(0)
                    if not brel_loaded[0]:
                        nc.scalar.dma_start(brel[:], brel_d[:])
                        brel_loaded[0] = True
                for bi in range(nblk_b + 3):
                    if not prefetch_at_end:
                        emit_prefetch(bi)
                    if not brel_loaded[0]:
                        nc.scalar.dma_start(brel[:], brel_d[:])
                        brel_loaded[0] = True
                    if bi < nblk_b:
                        for q in range(4):
                            t = 4 * bi + q
                            if is_dev_block(bi):
                                xt = x8s[t // 4][:, t % 4, :]
                                xq = xtpsum.tile([128, D], BF16, tag="xtp")
                                for k in range(4):
                                    nc.tensor.transpose(
                                        xq[:, k * 128:(k + 1) * 128],
                                        xt[:, k * 128:(k + 1) * 128],
                                        identb[:],
                                    )
                                dst = xtss[bi][:, :, q * 128:(q + 1) * 128]
                                src = xq[:].rearrange("p (k r) -> p k r", k=4)
                                nc.vector.tensor_copy(dst, src)
                                copy_alt[0] += 1

                    # ---- stage 1 (block bi-1): W1 matmuls + tanh
                    b1i = bi - 1
                    if 0 <= b1i < nblk_b:
                        nb = 4 * 128
                        xts1 = xtss.pop(b1i)
                        dev1 = is_dev_block(b1i)
                        w1use = w1b if dev1 else w18
                        u = upsum.tile([H, 4 * 128], F32, tag="u")
                        for k in range(4):
                            nc.tensor.matmul(
                                u[:, 0:nb],
                                w1use[:, k, :],
                                xts1[:, k, 0:nb],
                                start=(k == 0),
                                stop=(k == 3),
                            )
                        th = thpool.tile([H, 4 * 128], BF16, tag="th")
                        # ship blocks run the MLP at fp8e3 with W1
                        # pre-scaled x32; tanh's input scale compensates
                        nc.scalar.activation(
                            th[:, 0:nb], u[:, 0:nb],
                            AF.Tanh, bias=b1s[:],
                            scale=(1.0 if dev1 else 1.0 / 32.0),
                        )
                        ths[b1i] = th

                    # ---- stage 2 (block bi-2): score matmul + exp + bounce
                    b2i = bi - 2
                    if 0 <= b2i < nblk_b:
                        nb = 4 * 128
                        th2 = ths.pop(b2i)
                        ep2 = epsum.tile([2, 4 * 128], F32, tag="e")
                        nc.tensor.matmul(
                            ep2[:, 0:nb], w2r[:], th2[:, 0:nb],
                            start=True, stop=True, skip_group_check=True,
                        )
                        g = b2i - grp_start[0]      # slot within group
                        if g == 0:
                            es1 = espool.tile([1, GB * 512], BF16,
                                              tag="es", name="es1")
                            ess[0] = es1
                        es1 = ess[0]
                        nc.scalar.activation(
                            es1[0:1, g * 512:g * 512 + nb], ep2[0:1, 0:nb],
                            AF.Exp, bias=b2s[0:1, :], scale=1.0,
                        )
                        pend_blocks.append(4 * b2i)
                        rem_after = nblk_b - 1 - b2i
                        gb_cur = GB if (tail_gb == 0
                                        or rem_after >= GB + 2) else tail_gb
                        last_of_group = (g == gb_cur - 1) or (b2i == nblk_b - 1)
                        if last_of_group:
                            gs = grp_start[0]
                            grp_start[0] = b2i + 1
                            gn = g * 512 + nb    # valid scalars in group
                            gt = (gn + 127) // 128
                            # departition e: [1, gn] -> [gt, 128] via a DRAM
                            # bounce with contiguous descriptors; the tiny
                            # PE transpose to [128, gt] is emitted lagged in
                            # emit_pass2_block so PE never waits on it
                            ed = dramp.tile([GB * 512], BF16, tag="ed")
                            nc.scalar.dma_start(ed[0:gn], es1[0:1, 0:gn])
                            em = e4pool.tile([GB * 4, 128], BF16, tag="em")
                            nc.scalar.dma_start(
                                em[0:gt, :],
                                ed[0:gn].rearrange("(a u) -> a u", a=gt),
                            )
                            del ess[0]
                            p2q.append(dict(blocks=pend_blocks, em=em,
                                            gt=gt, e4=None, enq=bi, start=gs))
                            pend_blocks = []

                    # ---- lagged pass2, two sub-stages one step apart:
                    # num/den matmuls for the block A-built last step, then
                    # A-builds (DVE) for the next block
                    if built_q:
                        emit_pass2_block(built_q.pop(0))
                    if p2q and bi - p2q[0]["enq"] >= LAG:
                        built_q.append(emit_abuild_block(p2q[0]))
                        if not p2q[0]["blocks"]:
                            p2q.pop(0)
                    if prefetch_at_end:
                        emit_prefetch(bi + 1)
                # drain remaining pass2 groups
                while p2q or built_q:
                    if built_q:
                        emit_pass2_block(built_q.pop(0))
                    if p2q:
                        built_q.append(emit_abuild_block(p2q[0]))
                        if not p2q[0]["blocks"]:
                            p2q.pop(0)

                dsb = fpool.tile([SEGS, 1], F32)
                nc.vector.tensor_scalar(dsb[:], denc[:, 0:1],
                                        1e-8, None, op0=OP.add)
                rec = fpool.tile([SEGS, 1], F32)
                nc.vector.reciprocal(rec[:], dsb[:])
                osb = fpool.tile([SEGS, D], F32)
                nc.vector.tensor_scalar(osb[:], num[:], rec[:], None,
                                        op0=OP.mult)
                nc.sync.dma_start(out_d[:], osb[:])

    nc.compile()
    return nc


_NC_CACHE = {}


def get_nc(tiles=TILES):
    if tiles not in _NC_CACHE:
        _NC_CACHE[tiles] = build_nc(tiles)
    return _NC_CACHE[tiles]


def make_in_maps(x, batch, W1, b1, W2, b2, tiles=TILES, n_cores=N_CORES):
    """Host-side sharding: segment-aligned fixed windows + relative ids,
    both bf16 layouts of each window, pre-arranged W1.

    The device loads xr with 8 rows per partition per DMA (one contiguous
    8KB descriptor per partition): window position j = 1024g + 128a + p
    holds original window row 1024g + 8p + a.  brel and xtb are built in
    this permuted order so everything downstream stays consistent."""
    x = np.ascontiguousarray(np.asarray(x, dtype=np.float32))
    batch = np.asarray(batch).astype(np.int64)
    W1 = np.asarray(W1, dtype=np.float32)
    b1 = np.asarray(b1, dtype=np.float32).reshape(H, 1)
    W2 = np.ascontiguousarray(
        np.repeat(np.asarray(W2, dtype=np.float32).reshape(H, 1), 2, axis=1)
    ).astype(ml_dtypes.bfloat16)
    b2v = float(np.asarray(b2, dtype=np.float32).reshape(-1)[0])
    b2a = np.full((128, 1), b2v, dtype=np.float32)
    # w1 rearranged: [p, k, h] = W1[128k + p, h]; w18 = fp8e3 of W1*32
    w1p = np.ascontiguousarray(W1.reshape(4, 128, H).transpose(1, 0, 2))
    w1r = w1p.astype(ml_dtypes.bfloat16)
    w18 = (w1p * 32.0).astype(ml_dtypes.float8_e3m4)
    # device constants: [identity | iota-row | ones] bf16
    cst = np.zeros((128, 257), dtype=ml_dtypes.bfloat16)
    cst[:, 0:128] = np.eye(128, dtype=np.float32)
    cst[:, 128:256] = np.arange(128, dtype=np.float32)[None, :]
    cst[:, 256] = 1.0

    n = x.shape[0]
    f_rows = tiles * 128
    nblk = tiles // 4
    # perm[j] = original window row at permuted position j
    perm = (np.arange(f_rows).reshape(-1, 128, 4)
            .transpose(0, 2, 1).reshape(f_rows))
    bounds = np.searchsorted(batch, np.arange(0, n_cores + 1) * SEGS)
    owned = np.diff(bounds)
    if owned.max() > f_rows:
        return None  # caller falls back
    xb = x.astype(ml_dtypes.bfloat16)
    pad_to = int(bounds[:-1].max() + f_rows)
    if pad_to > n:
        xb = np.concatenate(
            [xb, np.zeros((pad_to - n, D), ml_dtypes.bfloat16)], axis=0)
    in_maps = []
    for c in range(n_cores):
        o = int(bounds[c])
        xs = np.ascontiguousarray(xb[o:o + f_rows])
        xsp = xs[perm]
        # [blk, p, k, r] = xsp[512 blk + r, 128k + p]: 4KB contiguous per
        # partition per block
        xtb = np.ascontiguousarray(
            xsp.reshape(nblk, 512, 4, 128).transpose(0, 3, 2, 1)
        ).astype(ml_dtypes.float8_e3m4)
        nb = min(f_rows, n - o) if n > o else 0
        br = np.full(f_rows, -1.0, dtype=np.float32)
        br[:nb] = batch[o:o + nb].astype(np.float32) - c * SEGS
        brp = br[perm]
        brel2d = np.ascontiguousarray(brp.reshape(tiles, 128).T)
        in_maps.append({
            "xr": xs, "xtb": xtb, "brel": brel2d, "w1": w1r, "w2": W2,
            "b1": b1, "b2": b2a, "cst": cst, "w18": w18,
        })
    return in_maps


def _numpy_fallback(x, batch, W1, b1, W2, b2):
    x = np.asarray(x, dtype=np.float32)
    batch = np.asarray(batch).astype(np.int64)
    scores = np.tanh(x @ W1 + b1) @ W2 + b2
    scores = scores - scores.max()
    e = np.exp(scores)
    den = np.zeros((B, 1), np.float32)
    np.add.at(den, batch, e)
    w = e / (den[batch] + 1e-8)
    out = np.zeros((B, D), np.float32)
    np.add.at(out, batch, w * x)
    return out


_RUNNER = {}


def _make_runner(nc, n_cores):
    """Reusable jitted SPMD executable (no donation) so repeated kernel()
    calls skip NEFF/XLA recompilation."""
    import jax
    import concourse.mybir as mybir
    from jax.sharding import Mesh, PartitionSpec, NamedSharding
    from jax.experimental.shard_map import shard_map
    from concourse import bass2jax

    bass2jax.install_neuronx_cc_hook()
    partition_name = (nc.partition_id_tensor.name
                      if nc.partition_id_tensor else None)
    in_names, out_names, out_avals, zero_outs = [], [], [], []
    for alloc in nc.m.functions[0].allocations:
        if not isinstance(alloc, mybir.MemoryLocationSet):
            continue
        name = alloc.memorylocations[0].name
        if alloc.kind == "ExternalInput":
            if name != partition_name:
                in_names.append(name)
        elif alloc.kind == "ExternalOutput":
            shape = tuple(alloc.tensor_shape)
            dtype = mybir.dt.np(alloc.dtype)
            out_names.append(name)
            out_avals.append(jax.core.ShapedArray(shape, dtype))
            zero_outs.append(np.zeros(shape, dtype))
    n_params = len(in_names)
    all_in_names = list(in_names) + list(out_names)
    if partition_name is not None:
        all_in_names.append(partition_name)

    def _body(*args):
        operands = list(args)
        if partition_name is not None:
            operands.append(bass2jax.partition_id_tensor())
        outs = bass2jax._bass_exec_p.bind(
            *operands,
            out_avals=tuple(out_avals),
            in_names=tuple(all_in_names),
            out_names=tuple(out_names),
            lowering_input_output_aliases=(),
            sim_require_finite=True,
            sim_require_nnan=True,
            nc=nc,
        )
        return tuple(outs)

    devices = jax.devices()[:n_cores]
    mesh = Mesh(np.asarray(devices), ("core",))
    nspec = n_params + len(out_names)
    fn = jax.jit(
        shard_map(_body, mesh=mesh,
                  in_specs=(PartitionSpec("core"),) * nspec,
                  out_specs=(PartitionSpec("core"),) * len(out_names),
                  check_rep=False),
        keep_unused=True,
    )
    sharding = NamedSharding(mesh, PartitionSpec("core"))
    concat_zero = [
        np.zeros((n_cores * z.shape[0], *z.shape[1:]), z.dtype) for z in zero_outs
    ]
    zero_dev = [jax.device_put(a, sharding) for a in concat_zero]
    return dict(fn=fn, in_names=in_names, out_names=out_names,
                out_avals=out_avals, zero_dev=zero_dev, sharding=sharding)


def _run_fast(nc, in_maps, n_cores):
    import jax
    if "r" not in _RUNNER:
        _RUNNER["r"] = _make_runner(nc, n_cores)
    r = _RUNNER["r"]
    concat_in = [
        np.concatenate([np.asarray(in_maps[c][name]) for c in range(n_cores)],
                       axis=0)
        for name in r["in_names"]
    ]
    dev_in = [jax.device_put(a, r["sharding"]) for a in concat_in]
    outs = r["fn"](*dev_in, *r["zero_dev"])
    jax.block_until_ready(outs)
    return [
        {name: np.asarray(outs[i]).reshape(n_cores, *r["out_avals"][i].shape)[c]
         for i, name in enumerate(r["out_names"])}
        for c in range(n_cores)
    ]


def kernel(x, batch, W1, b1, W2, b2):
    x = np.asarray(x)
    batch = np.asarray(batch)
    if (x.shape != (262144, D) or batch.shape != (262144,)
            or np.asarray(W1).shape != (D, H)):
        return _numpy_fallback(x, batch, W1, b1, W2, b2)
    if np.any(batch[:-1] > batch[1:]):
        return _numpy_fallback(x, batch, W1, b1, W2, b2)
    in_maps = make_in_maps(x, batch, W1, b1, W2, b2)
    if in_maps is None:
        return _numpy_fallback(x, batch, W1, b1, W2, b2)
    nc = get_nc()
    try:
        res = _run_fast(nc, in_maps, N_CORES)
        return np.concatenate([res[c]["out"] for c in range(N_CORES)], axis=0)
    except Exception:
        from concourse.bass_utils import run_bass_kernel_spmd
        res = run_bass_kernel_spmd(nc, in_maps, list(range(N_CORES)))
        return np.concatenate(
            [res.results[c]["out"] for c in range(N_CORES)], axis=0)


if __name__ == "__main__":
    pass
